# revision 6
# baseline (speedup 1.0000x reference)
"""Trainium2 Bass kernel for Jacobi-KAN layer.

y[b,o] = sum_{i,d} P_d(tanh(x[b,i])) * C[i,o,d],  B=262144, I=O=128, D+1=9,
Jacobi polynomials with a=b=1.

Strategy (pure data parallel over batch, 8 cores):
 - Host re-expresses the degree-8 Jacobi basis in the product basis
   {1, t, w, t*w, w^2, t*w^2, w^3, t*w^3, w^4} with w = 2t^2-1 (all values in
   [-1,1]); folds the 9x9 change of basis into the coefficient tensor in
   float64. The constant plane's contribution is a per-output bias
   c0[o] = sum_i C'[i,o,0], added during the PSUM->SBUF copy, so the PE only
   runs 8 accumulating matmuls per output tile instead of 9.
 - Host pre-transposes each x shard to (128, 32768) fp16 so the contraction
   axis i lands on SBUF partitions with perfectly contiguous DMA.
 - Device, per 2048-col chunk (skewed pipeline, tanh one chunk ahead):
     ACT: t = tanh(x); w2 = Square(2v-1) [fused input affine]; w4 = Square(w2)
     DVE: v = t*t; w = 2v-1 (tensor_scalar); tw; w3 = w*w2; tw2
     GPS: tw3 = t*w3; and per 512-group y = psum + c0 (fp16 out)
 - PE: per 512-column group, 8 accumulating fp16 matmuls with C_d stationary
   (128x128) and basis plane moving (128x512) -> PSUM y^T (o,b) fp32.
 - Host transposes y^T back on gather.
"""

import sys

for _p in ("/opt/trn_rl_repo", "/opt/trn_rl_repo/concourse"):
    if _p not in sys.path:
        sys.path.insert(0, _p)

import numpy as np

import concourse.bacc as bacc
import concourse.bass as bass
import concourse.mybir as mybir
from concourse.bass_utils import run_bass_kernel_spmd
from concourse.tile import TileContext

P = 128
N_CORES = 8
B_TOTAL = 262144
B_CORE = B_TOTAL // N_CORES        # 32768
ND = 9                             # number of basis functions
NMM = 8                            # matmul planes (constant plane folded out)
CHUNK = 2048                       # elementwise chunk (free dim)
NCHUNKS = B_CORE // CHUNK          # 16
GROUP = 512                        # matmul moving free dim
GROUPS_PER_CHUNK = CHUNK // GROUP  # 4

F16 = mybir.dt.float16
F32 = mybir.dt.float32
AF = mybir.ActivationFunctionType
OP = mybir.AluOpType


def _basis_transform():
    """9x9 float64 matrix T with C'[i,o,j] = sum_d C[i,o,d] * T[d,j] such that
    sum_j C'_j * basis_j(t) == sum_d C_d * JacobiP_d(t) for the basis
    [1, t, w, t*w, w^2, t*w^2, w^3, t*w^3, w^4], w = 2t^2-1."""
    import numpy.polynomial.polynomial as NP

    a_, b_ = 1.0, 1.0
    polys = [np.array([1.0]), np.array([0.0, 2.0])]
    for i in range(2, ND):
        Ai = (2 * i + a_ + b_ - 1) * (2 * i + a_ + b_) / (2 * i * (i + a_ + b_))
        Bi = (2 * i + a_ + b_ - 1) * (a_ ** 2 - b_ ** 2) / (
            2 * i * (i + a_ + b_) * (2 * i + a_ + b_ - 2))
        Ci = -2 * (i + a_ - 1) * (i + b_ - 1) * (2 * i + a_ + b_) / (
            2 * i * (i + a_ + b_) * (2 * i + a_ + b_ - 2))
        p = NP.polyadd(NP.polymul([Bi, Ai], polys[i - 1]),
                       NP.polymul([Ci], polys[i - 2]))
        polys.append(p)
    Jm = np.zeros((ND, ND))
    for d, p in enumerate(polys):
        Jm[d, :len(p)] = p

    t = np.array([0.0, 1.0])
    w = np.array([-1.0, 0.0, 2.0])
    w2 = NP.polymul(w, w)
    w3 = NP.polymul(w, w2)
    w4 = NP.polymul(w2, w2)
    basis = [np.array([1.0]), t, w, NP.polymul(t, w), w2, NP.polymul(t, w2),
             w3, NP.polymul(t, w3), w4]
    Bm = np.zeros((ND, ND))
    for j, p in enumerate(basis):
        Bm[j, :len(p)] = p
    return Jm @ np.linalg.inv(Bm)


def _build_module():
    nc = bacc.Bacc(trn_type="TRN2")
    xt = nc.dram_tensor("xt", [P, B_CORE], F16, kind="ExternalInput")
    cw = nc.dram_tensor("cw", [P, NMM * P], F16, kind="ExternalInput")
    c0 = nc.dram_tensor("c0", [P, 1], F32, kind="ExternalInput")
    yt = nc.dram_tensor("yt", [P, B_CORE], F16, kind="ExternalOutput")

    with TileContext(nc) as tc:
        with (
            tc.tile_pool(name="const", bufs=1) as const_pool,
            tc.tile_pool(name="xin", bufs=2) as xin_pool,
            tc.tile_pool(name="bas", bufs=2) as bas_pool,
            tc.tile_pool(name="yout", bufs=8) as yout_pool,
            tc.tile_pool(name="psum", bufs=8, space="PSUM") as psum_pool,
        ):
            cw_sb = const_pool.tile([P, NMM * P], F16)
            nc.sync.dma_start(cw_sb[:], cw[:, :])
            c0_sb = const_pool.tile([P, 1], F32)
            nc.sync.dma_start(c0_sb[:], c0[:, :])
            negone = const_pool.tile([P, 1], F32)
            nc.vector.memset(negone[:], -1.0)

            tiles = [None] * NCHUNKS

            def emit_load_tanh(c):
                xin = xin_pool.tile([P, CHUNK], F16)
                nc.sync.dma_start(xin[:], xt[:, c * CHUNK:(c + 1) * CHUNK])
                t = bas_pool.tile([P, CHUNK], F16)
                nc.scalar.activation(t[:], xin[:], AF.Tanh)
                tiles[c] = t

            def emit_rest(c):
                t = tiles[c]
                v = bas_pool.tile([P, CHUNK], F16)
                w = bas_pool.tile([P, CHUNK], F16)
                w2 = bas_pool.tile([P, CHUNK], F16)
                w3 = bas_pool.tile([P, CHUNK], F16)
                w4 = bas_pool.tile([P, CHUNK], F16)
                tw = bas_pool.tile([P, CHUNK], F16)
                tw2 = bas_pool.tile([P, CHUNK], F16)
                tw3 = bas_pool.tile([P, CHUNK], F16)

                nc.vector.tensor_tensor(v[:], t[:], t[:], OP.mult)
                nc.scalar.activation(w2[:], v[:], AF.Square,
                                     bias=negone[:, 0:1], scale=2.0)
                nc.vector.tensor_scalar(w[:], v[:], 2.0, -1.0,
                                        OP.mult, OP.add)
                nc.scalar.activation(w4[:], w2[:], AF.Square)
                nc.vector.tensor_tensor(tw[:], t[:], w[:], OP.mult)
                nc.vector.tensor_tensor(w3[:], w[:], w2[:], OP.mult)
                nc.vector.tensor_tensor(tw2[:], t[:], w2[:], OP.mult)
                nc.gpsimd.tensor_tensor(tw3[:], t[:], w3[:], OP.mult)

                planes = [t, w, tw, w2, tw2, w3, w4, tw3]
                for g in range(GROUPS_PER_CHUNK):
                    off = g * GROUP
                    acc = psum_pool.tile([P, GROUP], F32)
                    for j in range(NMM):
                        nc.tensor.matmul(
                            acc[:], cw_sb[:, j * P:(j + 1) * P],
                            planes[j][:, off:off + GROUP],
                            start=(j == 0), stop=(j == NMM - 1))
                    yo = yout_pool.tile([P, GROUP], F16)
                    nc.vector.tensor_scalar(yo[:], acc[:], c0_sb[:, 0:1],
                                            None, OP.add)
                    col = c * CHUNK + off
                    nc.sync.dma_start(yt[:, col:col + GROUP], yo[:])

            for c in range(NCHUNKS + 1):
                if c < NCHUNKS:
                    emit_load_tanh(c)
                if c >= 1:
                    emit_rest(c - 1)

    # TRN2 allows at most one sync wait per instruction; split multi-wait
    # instructions into event-semaphore chains (normally done in
    # Bacc.compile(), which the bass2jax serialization path does not run).
    from concourse import inst_simplify

    nc.insert_bir_kernel_barrier_sem_inc()
    nc.move_matmul_waits_to_ldweights()
    nc.generate_event_semaphores()
    nc.remove_dead_instructions_after_branch()
    nc.validate_blocks()
    nc.dce_regs()
    nc.thread_jumps()
    nc.remove_dead_blocks()
    nc.remove_dead_allocations()
    nc.verify_switch_hints()
    nc.alloc_regs()
    inst_simplify.simplify(nc)
    nc.fuse_regops()
    nc.fuse_blocks()
    nc.replace_nops_with_events()
    for engine in nc.engines:
        nc.fuse_nops(engine)
    nc.remove_dead_nops()
    nc.remove_dangling_data()
    nc.generate_event_semaphores()
    return nc


_NC_CACHE = None


def _make_in_maps(x: np.ndarray, jacobi_coeffs: np.ndarray) -> list:
    x = np.asarray(x)
    C = np.asarray(jacobi_coeffs)

    T = _basis_transform()
    Cp = np.einsum("iod,dj->ioj", C.astype(np.float64), T)
    # planes for matmul: [t, w, tw, w2, tw2, w3, tw3, w4] = basis idx
    # [1, 2, 3, 4, 5, 6, 7, 8] reordered to match device plane order.
    order = [1, 2, 3, 4, 5, 6, 8, 7]  # t, w, tw, w2, tw2, w3, w4(idx8), tw3(idx7)
    # device planes list: [t, w, tw, w2, tw2, w3, w4, tw3]
    cw = np.ascontiguousarray(
        Cp[:, :, order].transpose(0, 2, 1).reshape(P, NMM * P)
    ).astype(np.float16)
    c0 = np.ascontiguousarray(
        Cp[:, :, 0].sum(axis=0).reshape(P, 1)).astype(np.float32)

    in_maps = []
    for k in range(N_CORES):
        shard = x[k * B_CORE:(k + 1) * B_CORE].astype(np.float16)
        in_maps.append({
            "xt": np.ascontiguousarray(shard.T),
            "cw": cw,
            "c0": c0,
        })
    return in_maps


def kernel(x: np.ndarray, jacobi_coeffs: np.ndarray) -> np.ndarray:
    global _NC_CACHE
    in_maps = _make_in_maps(x, jacobi_coeffs)

    if _NC_CACHE is None:
        _NC_CACHE = _build_module()

    res = run_bass_kernel_spmd(_NC_CACHE, in_maps, core_ids=list(range(N_CORES)))
    out = np.concatenate(
        [np.asarray(r["yt"]).astype(np.float32).T for r in res.results], axis=0)
    return np.ascontiguousarray(out)
